# revision 2
# baseline (speedup 1.0000x reference)
"""Trainium2 Bass kernel for nn_ConditionedCategorical (segment_reduce).

Computes, for inputs x_labels [N] (values in [0,16)), y_labels [N] (values in
[0,32)), posterior_estimate [N, 16] fp32:

    numerator[k, y, :] = eps + sum_{n: x_n=k, y_n=y} posterior[n, :]
    out = numerator / numerator.sum(axis=1, keepdims=True)      # [16, 32, 16]

Strategy (data-parallel over 8 NeuronCores, N/8 rows per core):
  - rows are laid out partition-major: row n = p*NT + t maps to SBUF
    (partition p, tile t); each DMA line is contiguous per partition.
  - per 128-row tile: build a [128, 512] fp16 one-hot of the composite label
    (x*32 + y) on the VectorE via a single tensor_scalar(is_equal) against a
    resident iota row, then accumulate numerator[c, s] (+)= post_tile.T @ onehot
    on the TensorE into a single PSUM bank ([16, 512] fp32) across all tiles.
  - AllReduce the [16, 512] partial across the 8 cores, add eps, normalize
    over Y on-device, and DMA the [16, 32, 16] result out (every core emits
    the full output; core 0's copy is returned).
"""

import numpy as np

import concourse.bass as bass
import concourse.tile as tile
from concourse import bacc, mybir
from concourse.bass_utils import run_bass_kernel_spmd

K, Y, C = 16, 32, 16
S = K * Y  # 512 composite buckets
EPS = 1e-8
NCORES = 8
P = 128

f32 = mybir.dt.float32
f16 = mybir.dt.float16
i32 = mybir.dt.int32
i16 = mybir.dt.int16

# last BassKernelResults (for test harness inspection)
last_results = None


def build_nc(nt: int, st: int, repeat: int = 1, single_core: bool = False,
             no_mm: bool = False, fixed_oh: bool = False, psum_banks: int = 1,
             oh_bufs: int = 6, oh_group: int = 1):
    """Build the SPMD program. nt = 128-row tiles per core, st = tiles per
    posterior supertile DMA. repeat re-runs the main loop (PSUM restarts each
    pass, so the result is unchanged) — used for steady-state timing.
    single_core skips the collective (for TimelineSim cost modelling).
    no_mm / fixed_oh are timing-isolation variants (wrong results)."""
    assert nt % st == 0
    assert st % oh_group == 0
    ndev = 1 if single_core else NCORES
    nc = bacc.Bacc("TRN2", target_bir_lowering=False, debug=False, num_devices=ndev)

    xl = nc.declare_dram_parameter("xl", [P, nt], i32, isOutput=False)
    yl = nc.declare_dram_parameter("yl", [P, nt], i32, isOutput=False)
    post = nc.declare_dram_parameter("post", [P, nt * C], f32, isOutput=False)
    out = nc.declare_dram_parameter("out", [K, Y, C], f32, isOutput=True)

    with tile.TileContext(nc) as tc:
        with (
            tc.tile_pool(name="setup", bufs=1) as setup,
            tc.tile_pool(name="persist", bufs=1) as persist,
            tc.tile_pool(name="postf32", bufs=3) as postf32_pool,
            tc.tile_pool(name="postf16", bufs=3) as postf16_pool,
            tc.tile_pool(name="oh", bufs=oh_bufs) as oh_pool,
            tc.tile_pool(name="acc", bufs=1, space="PSUM") as acc_pool,
            tc.tile_pool(name="epi", bufs=1) as epi,
            tc.tile_pool(name="dram", bufs=1, space="DRAM") as dram,
        ):
            # --- setup: iota row + composite labels ---
            iota_i = setup.tile([P, S], i16)
            nc.gpsimd.iota(iota_i[:], pattern=[[1, S]], base=0, channel_multiplier=0)
            iota_f = persist.tile([P, S], f16)
            nc.vector.tensor_copy(iota_f[:], iota_i[:])

            xl_sb = setup.tile([P, nt], i32)
            nc.sync.dma_start(xl_sb[:], xl[:])
            yl_sb = setup.tile([P, nt], i32)
            nc.sync.dma_start(yl_sb[:], yl[:])
            xf = setup.tile([P, nt], f32)
            nc.vector.tensor_copy(xf[:], xl_sb[:])
            yf = setup.tile([P, nt], f32)
            nc.vector.tensor_copy(yf[:], yl_sb[:])
            comp = persist.tile([P, nt], f32)
            # comp = x*32 + y  (exact: values < 512)
            nc.vector.scalar_tensor_tensor(
                comp[:], xf[:], float(Y), yf[:],
                mybir.AluOpType.mult, mybir.AluOpType.add,
            )

            # --- main loop: one-hot + matmul-accumulate ---
            accs = [acc_pool.tile([C, S], f32, name=f"acc{b}", tag=f"acc{b}")
                    for b in range(psum_banks)]
            acc = accs[0]
            fixed_oh_t = None
            if fixed_oh:
                fixed_oh_t = persist.tile([P, S], f16)
                nc.vector.memset(fixed_oh_t[:], 0.0)
            n_super = nt // st
            for rep in range(repeat):
                for sti in range(n_super):
                    pf32 = postf32_pool.tile([P, st * C], f32)
                    nc.sync.dma_start(pf32[:], post[:, sti * st * C:(sti + 1) * st * C])
                    pf16 = postf16_pool.tile([P, st * C], f16)
                    nc.scalar.copy(pf16[:], pf32[:])
                    for j in range(st):
                        t = sti * st + j
                        g = j % oh_group
                        if fixed_oh:
                            oh = fixed_oh_t
                        else:
                            if g == 0:
                                ohg = oh_pool.tile([P, oh_group * S], f16,
                                                   name="ohg", tag="ohg")
                            oh = ohg[:, g * S:(g + 1) * S]
                            nc.vector.tensor_scalar(
                                oh, iota_f[:], comp[:, t:t + 1], None,
                                mybir.AluOpType.is_equal,
                            )
                        if not no_mm:
                            nc.tensor.matmul(
                                accs[t % psum_banks][:],
                                lhsT=pf16[:, j * C:(j + 1) * C],
                                rhs=oh if fixed_oh else ohg[:, g * S:(g + 1) * S],
                                start=(t < psum_banks and rep == 0),
                                stop=(t >= nt - psum_banks and rep == repeat - 1),
                            )
            if no_mm:
                for b in range(psum_banks):
                    nc.vector.memset(accs[b][:], 0.0)
            # --- epilogue: allreduce, eps, normalize over Y, emit ---
            accsb = epi.tile([C, S], f32)
            nc.vector.tensor_copy(accsb[:], acc[:])
            for b in range(1, psum_banks):
                nc.vector.tensor_tensor(accsb[:], accsb[:], accs[b][:],
                                        op=mybir.AluOpType.add)
            cc_in = dram.tile([C, S], f32)
            nc.sync.dma_start(cc_in[:], accsb[:])
            num = epi.tile([C, S], f32)
            if single_core:
                nc.sync.dma_start(num[:], cc_in[:])
            else:
                cc_out = nc.dram_tensor("cc_out", [C, S], f32, addr_space="Shared")
                nc.gpsimd.collective_compute(
                    "AllReduce",
                    mybir.AluOpType.add,
                    replica_groups=[list(range(NCORES))],
                    ins=[cc_in[:]],
                    outs=[cc_out[:]],
                )
                nc.sync.dma_start(num[:], cc_out[:])
            nc.vector.tensor_scalar(
                num[:], num[:], EPS, None, mybir.AluOpType.add,
            )
            den = epi.tile([C, K], f32)
            nc.vector.tensor_reduce(
                den[:],
                num[:].rearrange("c (k y) -> c k y", y=Y),
                axis=mybir.AxisListType.X,
                op=mybir.AluOpType.add,
            )
            rec = epi.tile([C, K], f32)
            nc.vector.reciprocal(rec[:], den[:])
            norm = epi.tile([C, S], f32)
            nc.vector.tensor_tensor(
                norm[:].rearrange("c (k y) -> c k y", y=Y),
                num[:].rearrange("c (k y) -> c k y", y=Y),
                rec[:].unsqueeze(2).broadcast_to((C, K, Y)),
                op=mybir.AluOpType.mult,
            )
            # out[k, y, c] = norm[c, k*Y + y]
            nc.sync.dma_start(
                out[:].rearrange("k y c -> c k y"),
                norm[:].rearrange("c (k y) -> c k y", y=Y),
            )

    nc.compile()
    return nc


_nc_cache = {}


def _get_nc(nt: int, st: int):
    key = (nt, st)
    if key not in _nc_cache:
        _nc_cache[key] = build_nc(nt, st)
    return _nc_cache[key]


def make_in_maps(x_labels, y_labels, posterior_estimate, nt):
    nloc = nt * P
    xi = np.ascontiguousarray(x_labels.astype(np.int32))
    yi = np.ascontiguousarray(y_labels.astype(np.int32))
    in_maps = []
    for i in range(NCORES):
        sl = slice(i * nloc, (i + 1) * nloc)
        in_maps.append({
            "xl": xi[sl].reshape(P, nt),
            "yl": yi[sl].reshape(P, nt),
            "post": posterior_estimate[sl].reshape(P, nt * C),
        })
    return in_maps


def kernel(x_labels, y_labels, posterior_estimate, _trace=False, _tmpdir=None):
    global last_results
    x_labels = np.asarray(x_labels)
    y_labels = np.asarray(y_labels)
    posterior_estimate = np.ascontiguousarray(
        np.asarray(posterior_estimate, dtype=np.float32)
    )
    n = x_labels.shape[0]
    assert n % (NCORES * P) == 0
    nloc = n // NCORES
    nt = nloc // P
    st = 64 if nt % 64 == 0 else (8 if nt % 8 == 0 else 1)

    nc = _get_nc(nt, st)

    in_maps = make_in_maps(x_labels, y_labels, posterior_estimate, nt)

    kwargs = {}
    if _trace:
        kwargs.update(trace=True, tmpdir=_tmpdir)
    res = run_bass_kernel_spmd(nc, in_maps, list(range(NCORES)), **kwargs)
    last_results = res
    return res.results[0]["out"]



# revision 3
# speedup vs baseline: 1.0839x; 1.0839x over previous
"""Trainium2 Bass kernel for nn_ConditionedCategorical (segment_reduce).

Computes, for inputs x_labels [N] (values in [0,16)), y_labels [N] (values in
[0,32)), posterior_estimate [N, 16] fp32:

    numerator[k, y, :] = eps + sum_{n: x_n=k, y_n=y} posterior[n, :]
    out = numerator / numerator.sum(axis=1, keepdims=True)      # [16, 32, 16]

Strategy (data-parallel over 8 NeuronCores, N/8 rows per core):
  - rows are laid out partition-major: row n = p*NT + t maps to SBUF
    (partition p, tile t); each DMA line is contiguous per partition.
  - per 128-row tile: build a [128, 512] fp16 one-hot of the composite label
    (x*32 + y) on the VectorE via a single tensor_scalar(is_equal) against a
    resident iota row, then accumulate numerator[c, s] (+)= post_tile.T @ onehot
    on the TensorE into a single PSUM bank ([16, 512] fp32) across all tiles.
  - AllReduce the [16, 512] partial across the 8 cores, add eps, normalize
    over Y on-device, and DMA the [16, 32, 16] result out (every core emits
    the full output; core 0's copy is returned).
"""

import numpy as np

import concourse.bass as bass
import concourse.tile as tile
from concourse import bacc, mybir
from concourse.bass_utils import run_bass_kernel_spmd

K, Y, C = 16, 32, 16
S = K * Y  # 512 composite buckets
EPS = 1e-8
NCORES = 8
P = 128

f32 = mybir.dt.float32
f16 = mybir.dt.float16
i32 = mybir.dt.int32
i16 = mybir.dt.int16

# last BassKernelResults (for test harness inspection)
last_results = None


def build_nc(nt: int, st: int, repeat: int = 1, single_core: bool = False,
             no_mm: bool = False, fixed_oh: bool = False, psum_banks: int = 1,
             oh_bufs: int = 6, oh_group: int = 1, act_tiles: int = 9):
    """Build the SPMD program. nt = 128-row tiles per core, st = tiles per
    posterior supertile DMA. repeat re-runs the main loop (PSUM restarts each
    pass, so the result is unchanged) — used for steady-state timing.
    single_core skips the collective (for TimelineSim cost modelling).
    no_mm / fixed_oh are timing-isolation variants (wrong results).
    act_tiles: tiles per supertile whose one-hot is built on the ScalarE
    (Square+Relu LUT trick) to offload the VectorE bottleneck."""
    assert nt % st == 0
    assert st % oh_group == 0
    ndev = 1 if single_core else NCORES
    nc = bacc.Bacc("TRN2", target_bir_lowering=False, debug=False, num_devices=ndev)

    xl = nc.declare_dram_parameter("xl", [P, nt], i32, isOutput=False)
    yl = nc.declare_dram_parameter("yl", [P, nt], i32, isOutput=False)
    post = nc.declare_dram_parameter("post", [P, nt * C], f32, isOutput=False)
    out = nc.declare_dram_parameter("out", [K, Y, C], f32, isOutput=True)

    # Spread the ScalarE-built tiles evenly through each supertile.
    act_set = set()
    if act_tiles > 0:
        act_set = {(i * st) // act_tiles + (st // (2 * act_tiles))
                   for i in range(act_tiles)}

    with tile.TileContext(nc) as tc:
        with (
            tc.tile_pool(name="setup", bufs=1) as setup,
            tc.tile_pool(name="persist", bufs=1) as persist,
            tc.tile_pool(name="postf32", bufs=3) as postf32_pool,
            tc.tile_pool(name="postf16", bufs=3) as postf16_pool,
            tc.tile_pool(name="oh", bufs=oh_bufs) as oh_pool,
            tc.tile_pool(name="sq", bufs=3) as sq_pool,
            tc.tile_pool(name="oha", bufs=3) as oha_pool,
            tc.tile_pool(name="acc", bufs=1, space="PSUM") as acc_pool,
            tc.tile_pool(name="epi", bufs=1) as epi,
            tc.tile_pool(name="dram", bufs=1, space="DRAM") as dram,
        ):
            # --- setup: iota row + composite labels ---
            iota_i = setup.tile([P, S], i16)
            nc.gpsimd.iota(iota_i[:], pattern=[[1, S]], base=0, channel_multiplier=0)
            iota_f = persist.tile([P, S], f16)
            nc.vector.tensor_copy(iota_f[:], iota_i[:])

            xl_sb = setup.tile([P, nt], i32)
            nc.sync.dma_start(xl_sb[:], xl[:])
            yl_sb = setup.tile([P, nt], i32)
            nc.sync.dma_start(yl_sb[:], yl[:])
            xf = setup.tile([P, nt], f32)
            nc.vector.tensor_copy(xf[:], xl_sb[:])
            yf = setup.tile([P, nt], f32)
            nc.vector.tensor_copy(yf[:], yl_sb[:])
            comp = persist.tile([P, nt], f32)
            # comp = x*32 + y  (exact: values < 512)
            nc.vector.scalar_tensor_tensor(
                comp[:], xf[:], float(Y), yf[:],
                mybir.AluOpType.mult, mybir.AluOpType.add,
            )
            # comp_n32 = -comp/32, bias input for the ScalarE one-hot path
            comp_n32 = persist.tile([P, nt], f32)
            if act_tiles > 0:
                nc.vector.tensor_scalar(
                    comp_n32[:], comp[:], -1.0 / 32.0, None,
                    mybir.AluOpType.mult,
                )

            # --- main loop: one-hot + matmul-accumulate ---
            accs = [acc_pool.tile([C, S], f32, name=f"acc{b}", tag=f"acc{b}")
                    for b in range(psum_banks)]
            acc = accs[0]
            fixed_oh_t = None
            if fixed_oh:
                fixed_oh_t = persist.tile([P, S], f16)
                nc.vector.memset(fixed_oh_t[:], 0.0)
            n_super = nt // st
            for rep in range(repeat):
                for sti in range(n_super):
                    pf32 = postf32_pool.tile([P, st * C], f32)
                    nc.sync.dma_start(pf32[:], post[:, sti * st * C:(sti + 1) * st * C])
                    pf16 = postf16_pool.tile([P, st * C], f16)
                    nc.scalar.copy(pf16[:], pf32[:])
                    for j in range(st):
                        t = sti * st + j
                        g = j % oh_group
                        if fixed_oh:
                            oh = fixed_oh_t
                            mm_rhs = oh[:]
                        elif j in act_set:
                            # ScalarE one-hot: Relu(1 - 1024*((iota - comp)/32)^2)
                            sq = sq_pool.tile([P, S], f16, name="sq", tag="sq")
                            nc.scalar.activation(
                                sq[:], iota_f[:],
                                mybir.ActivationFunctionType.Square,
                                bias=comp_n32[:, t:t + 1], scale=1.0 / 32.0,
                            )
                            oha = oha_pool.tile([P, S], f16, name="oha", tag="oha")
                            nc.scalar.activation(
                                oha[:], sq[:],
                                mybir.ActivationFunctionType.Relu,
                                bias=1.0, scale=-1024.0,
                            )
                            mm_rhs = oha[:]
                        else:
                            if g == 0:
                                ohg = oh_pool.tile([P, oh_group * S], f16,
                                                   name="ohg", tag="ohg")
                            oh = ohg[:, g * S:(g + 1) * S]
                            nc.vector.tensor_scalar(
                                oh, iota_f[:], comp[:, t:t + 1], None,
                                mybir.AluOpType.is_equal,
                            )
                            mm_rhs = ohg[:, g * S:(g + 1) * S]
                        if not no_mm:
                            nc.tensor.matmul(
                                accs[t % psum_banks][:],
                                lhsT=pf16[:, j * C:(j + 1) * C],
                                rhs=mm_rhs,
                                start=(t < psum_banks and rep == 0),
                                stop=(t >= nt - psum_banks and rep == repeat - 1),
                            )
            if no_mm:
                for b in range(psum_banks):
                    nc.vector.memset(accs[b][:], 0.0)
            # --- epilogue: allreduce, eps, normalize over Y, emit ---
            accsb = epi.tile([C, S], f32)
            nc.vector.tensor_copy(accsb[:], acc[:])
            for b in range(1, psum_banks):
                nc.vector.tensor_tensor(accsb[:], accsb[:], accs[b][:],
                                        op=mybir.AluOpType.add)
            cc_in = dram.tile([C, S], f32)
            nc.sync.dma_start(cc_in[:], accsb[:])
            num = epi.tile([C, S], f32)
            if single_core:
                nc.sync.dma_start(num[:], cc_in[:])
            else:
                cc_out = nc.dram_tensor("cc_out", [C, S], f32, addr_space="Shared")
                nc.gpsimd.collective_compute(
                    "AllReduce",
                    mybir.AluOpType.add,
                    replica_groups=[list(range(NCORES))],
                    ins=[cc_in[:]],
                    outs=[cc_out[:]],
                )
                nc.sync.dma_start(num[:], cc_out[:])
            nc.vector.tensor_scalar(
                num[:], num[:], EPS, None, mybir.AluOpType.add,
            )
            den = epi.tile([C, K], f32)
            nc.vector.tensor_reduce(
                den[:],
                num[:].rearrange("c (k y) -> c k y", y=Y),
                axis=mybir.AxisListType.X,
                op=mybir.AluOpType.add,
            )
            rec = epi.tile([C, K], f32)
            nc.vector.reciprocal(rec[:], den[:])
            norm = epi.tile([C, S], f32)
            nc.vector.tensor_tensor(
                norm[:].rearrange("c (k y) -> c k y", y=Y),
                num[:].rearrange("c (k y) -> c k y", y=Y),
                rec[:].unsqueeze(2).broadcast_to((C, K, Y)),
                op=mybir.AluOpType.mult,
            )
            # out[k, y, c] = norm[c, k*Y + y]
            nc.sync.dma_start(
                out[:].rearrange("k y c -> c k y"),
                norm[:].rearrange("c (k y) -> c k y", y=Y),
            )

    nc.compile()
    return nc


_nc_cache = {}


def _get_nc(nt: int, st: int):
    key = (nt, st)
    if key not in _nc_cache:
        _nc_cache[key] = build_nc(nt, st)
    return _nc_cache[key]


def make_in_maps(x_labels, y_labels, posterior_estimate, nt):
    nloc = nt * P
    xi = np.ascontiguousarray(x_labels.astype(np.int32))
    yi = np.ascontiguousarray(y_labels.astype(np.int32))
    in_maps = []
    for i in range(NCORES):
        sl = slice(i * nloc, (i + 1) * nloc)
        in_maps.append({
            "xl": xi[sl].reshape(P, nt),
            "yl": yi[sl].reshape(P, nt),
            "post": posterior_estimate[sl].reshape(P, nt * C),
        })
    return in_maps


def kernel(x_labels, y_labels, posterior_estimate, _trace=False, _tmpdir=None):
    global last_results
    x_labels = np.asarray(x_labels)
    y_labels = np.asarray(y_labels)
    posterior_estimate = np.ascontiguousarray(
        np.asarray(posterior_estimate, dtype=np.float32)
    )
    n = x_labels.shape[0]
    assert n % (NCORES * P) == 0
    nloc = n // NCORES
    nt = nloc // P
    st = 64 if nt % 64 == 0 else (8 if nt % 8 == 0 else 1)

    nc = _get_nc(nt, st)

    in_maps = make_in_maps(x_labels, y_labels, posterior_estimate, nt)

    kwargs = {}
    if _trace:
        kwargs.update(trace=True, tmpdir=_tmpdir)
    res = run_bass_kernel_spmd(nc, in_maps, list(range(NCORES)), **kwargs)
    last_results = res
    return res.results[0]["out"]

